# revision 52
# baseline (speedup 1.0000x reference)
"""Trainium2 Bass kernel for nn_BatchedModelManifoldGeodesicFlow.

Math (validated against the reference):
  G = J^T J + eps*I is symmetric => the Christoffel contraction collapses:
    einsum('bijk,bj,bk->bi', Gamma, v, v) = 0.5 * einsum('bijk,bj,bk->bi', dG, v, v)
  With f = tanh(x@W1+b1)@W2+b2, s = tanh(z), d1 = 1-s^2, d2 = -2*s*d1:
    T_i := sum_jk dG[i,j,k] v_j v_k = (W1 @ g)_i,
       g = d2*p*(S@(d1*p)) + d1*(S@(d2*p*p)),  p = W1^T v,  S = W2@W2^T
    ||dG||_F^2 = 2*(<G1,G2> + sum_{o,o'} Y[:,(o,o')].Y[:,(o',o)])
       E = d1[:,None]*W2, C = d2[:,None]*W2, K = W1^T W1, K2 = K*K
       F = K@E, G1 = E^T F, G2 = C^T (K2@C), Y[:, o*O+o'] = W1@(C[:,o']*F[:,o])
    a = -0.5*T/(||dG||_F * ||v||);  out = concat(v, a - 0.1*dev)

Implementation notes:
  - Weight-only transforms on host, shipped bf16: W1^T, K = W1^T W1,
    K2 = K*K, S = W2@W2^T (kills the W2^T/u round-trip in the T path).
    Per-sample tensors (dev, x0, x1, v) shipped raw feature-major; x,
    |v|^2, -0.1*dev computed on device.
  - First DMA carries samples + W1 + b1 + W2 + scalar consts together
    (500ns floor per DMA) so the z matmul is gated only by the ~2.4us DMA
    pipe latency; K/S/K2+W1^T follow on the SP queue in dependency order.
    A tiny ACT-queue DMA anchors the tanh act-table load at t~300.
  - Norm pipeline (E, C, F, KC, mx, Y, termA/termB) in bf16 (PE bf16 is
    4x f32); T path f32 inputs with bf16 projections. Rel err ~1e-4.
  - A dummy 1x1 matmul at t~400 starts the PE p-state ramp (full clock
    needs 3us of history).
  - Engine/ISA rules honored: gpsimd(Pool) never touches PSUM and uses
    only TensorTensor/TensorCopy/partition ops; every DVE/ACT op reads at
    most one PSUM operand; no sqrt/pow - 1/(nf*|v|) is a quake-style
    rsqrt (bit-hack seed + one Newton step) in near-free [1,4] ops.
  - Output stays feature-major to the end; the store DMA performs the
    transpose with a strided DRAM access pattern.

Sharding: pure batch parallelism, B=32 over 8 cores (4 samples/core),
params replicated.
"""

import os
import sys

if "/opt/trn_rl_repo" not in sys.path:
    sys.path.insert(0, "/opt/trn_rl_repo")

import numpy as np

B, D, H, O = 32, 128, 256, 10
NCORES = 8
BC = B // NCORES  # 4 samples per core
OO = O * O  # 100

_PROGRAM = None


def _build_program():
    import concourse.bass as bass
    import concourse.bacc as bacc
    import concourse.tile as tile
    from concourse import mybir
    from concourse.masks import make_identity

    f32 = mybir.dt.float32
    bf16 = mybir.dt.bfloat16
    mult = mybir.AluOpType.mult
    add = mybir.AluOpType.add
    pow_ = mybir.AluOpType.pow
    AF = mybir.ActivationFunctionType

    nc = bacc.Bacc(None)
    # xw: cols 0:4 dev^T | 4:8 x0^T | 8:12 x1^T | 12:16 v^T | 16 t | 17 win
    #     | 18:274 W1 | 274:276 b1 (h-chunked) | 276:296 W2 (cols hc*10+o)
    xw_d = nc.declare_dram_parameter("xw", [D, 304], f32, isOutput=False)
    kb_d = nc.declare_dram_parameter("kb", [D, 512], bf16, isOutput=False)
    sb_d = nc.declare_dram_parameter("sb", [D, 512], bf16, isOutput=False)
    kw_d = nc.declare_dram_parameter("kw", [D, 768], bf16, isOutput=False)
    acc_d = nc.declare_dram_parameter("acc", [BC, D], f32, isOutput=True)

    with tile.TileContext(nc) as tc:
        with (
            tc.tile_pool(name="const", bufs=1) as const,
            tc.tile_pool(name="work", bufs=1) as work,
            tc.tile_pool(name="ps", bufs=1, space="PSUM") as ps,
        ):
            # ---------------- input DMAs (SP + ACT queues) ------------------
            xw_sb = const.tile([D, 304], f32)
            nc.sync.dma_start(out=xw_sb, in_=xw_d[:])
            # tiny ACT-queue DMA: anchors the tanh act-table load at t~300
            # (with no DMA on the ACT queue the scheduler couples the table
            # load to the first activation's DMA waits, delaying tanh ~1.3us);
            # its engine-busy window (~2.0-2.5us) ends before tanh needs ACT.
            anchor_sb = const.tile([1, 4], f32)
            nc.scalar.dma_start(out=anchor_sb, in_=xw_d[0:1, 0:4])
            kb_sb = const.tile([D, 512], bf16)
            nc.sync.dma_start(out=kb_sb, in_=kb_d[:])
            sb_sb = const.tile([D, 512], bf16)
            nc.sync.dma_start(out=sb_sb, in_=sb_d[:])
            kw_sb = const.tile([D, 768], bf16)
            nc.sync.dma_start(out=kw_sb, in_=kw_d[:])

            devt = xw_sb[:, 0:BC]
            x0t = xw_sb[:, BC : 2 * BC]
            x1t = xw_sb[:, 2 * BC : 3 * BC]
            vt = xw_sb[:, 3 * BC : 4 * BC]
            tcol = xw_sb[:, 16:17]
            wcol = xw_sb[:, 17:18]
            w1 = xw_sb[:, 18:274]
            b1 = xw_sb[:, 274:276]
            w2 = xw_sb[:, 276:296]
            magic_c = xw_sb[:, 296:297]
            oneb_c = xw_sb[:, 297:298]
            c15_c = xw_sb[:, 298:299]
            cnh_c = xw_sb[:, 299:300]
            npt_c = xw_sb[:, 300:301]
            k_bf = [kb_sb[:, 0:256], kb_sb[:, 256:512]]
            k2_bf = [kw_sb[:, 0:256], kw_sb[:, 256:512]]
            w1t_bf = [kw_sb[:, 512:640], kw_sb[:, 640:768]]
            s_bf = [sb_sb[:, 0:256], sb_sb[:, 256:512]]

            # ---------------- constants + PE warm-up ------------------------
            ones_c = const.tile([128, 1], f32)
            nc.vector.memset(ones_c, 1.0)
            twos_c = const.tile([128, 1], f32)
            nc.vector.memset(twos_c, 2.0)
            fours_c = const.tile([128, 1], f32)
            nc.vector.memset(fours_c, 4.0)

            # PSUM banks (packed; allocator is bank-granular):
            #  zq:  [D,40]  z/p (0:16) | q (16:32) | u rows 0:10 (32:40), warm 39
            #  fkT: [D,164] F (0:80) | KC (80:160) | T (160:164)
            #  g:   [O,80]  per-sample G1|G2
            #  tv:  [BC,258] T^T (0:128) | devneg^T (128:256) | vn | r4
            #  Y:   2x [D,200]
            ps_zq = ps.tile([D, 40], f32, tag="zq", bufs=1)
            ps_fb = ps.tile([D, 80], f32, tag="F", bufs=1)
            ps_kb = ps.tile([D, 80], f32, tag="KC", bufs=1)
            ps_Tm = ps.tile([D, BC], f32, tag="T", bufs=1)
            ps_gt = ps.tile([O, 2 * O * BC], f32, tag="g", bufs=1)
            ps_tv = ps.tile([1, 2 * BC], f32, tag="tv", bufs=1)

            # PE p-state ramp starter: tiny matmul as early as possible
            nc.tensor.matmul(
                ps_zq[0:1, 39:40], ones_c, ones_c[:, 0:1], start=True, stop=True,
                skip_group_check=True,
            )

            # ---------------- x = x0 + t*(x1-x0) + w*dev (Pool, tiny) -------
            # Pool supports only TensorTensor/TensorCopy: build x with TT ops
            # using stride-0 views of the t/window columns
            def _bcast(col, n):
                return bass.AP(tensor=col.tensor, offset=col.offset,
                               ap=[col.ap[0], [0, n]])

            dx = work.tile([D, BC], f32)
            nc.gpsimd.tensor_sub(dx, x1t, x0t)
            tdx = work.tile([D, BC], f32)
            nc.gpsimd.tensor_mul(tdx, dx, _bcast(tcol, BC))
            xa = work.tile([D, BC], f32)
            nc.gpsimd.tensor_add(xa, tdx, x0t)
            wdev = work.tile([D, BC], f32)
            nc.gpsimd.tensor_mul(wdev, devt, _bcast(wcol, BC))
            xf = work.tile([D, BC], f32)
            nc.gpsimd.tensor_add(xf, wdev, xa)

            # ---------------- z,p = W1^T @ [x|v]; tanh ----------------------
            # layout: z cols 0:8 (hc-major), p cols 8:16 (hc-major)
            # p (v) matmuls first: they don't wait on the xf chain
            for hc in range(2):
                w1c = w1[:, hc * 128 : (hc + 1) * 128]
                nc.tensor.matmul(ps_zq[:, 2 * BC + hc * BC : 2 * BC + (hc + 1) * BC], w1c, vt, start=True, stop=True, skip_group_check=True)
            for hc in range(2):
                w1c = w1[:, hc * 128 : (hc + 1) * 128]
                nc.tensor.matmul(ps_zq[:, hc * BC : (hc + 1) * BC], w1c, xf, start=True, stop=True, skip_group_check=True)
            s_act = work.tile([D, 2 * BC], f32)  # cols hc*BC+b
            for hc in range(2):
                nc.scalar.activation(
                    s_act[:, hc * BC : (hc + 1) * BC],
                    ps_zq[:, hc * BC : (hc + 1) * BC],
                    AF.Tanh,
                    bias=b1[:, hc : hc + 1],
                )
            # p (both hc) in one contiguous PSUM->SBUF copy (DVE: gpsimd may
            # not touch PSUM on real hardware)
            p_t = work.tile([D, 2 * BC], f32)
            nc.vector.tensor_copy(p_t, ps_zq[:, 2 * BC : 4 * BC])

            # ---------------- d1 = 1-s^2, d2 = -2*s*d1 (Pool) ---------------
            s2 = work.tile([D, 2 * BC], f32)
            d1 = work.tile([D, 2 * BC], f32)
            d2 = work.tile([D, 2 * BC], f32)
            sub = mybir.AluOpType.subtract
            for hc in range(2):
                hsl = slice(hc * BC, (hc + 1) * BC)
                nc.gpsimd.tensor_mul(s2[:, hsl], s_act[:, hsl], s_act[:, hsl])
                nc.gpsimd.tensor_tensor(d1[:, hsl], _bcast(ones_c, BC), s2[:, hsl], sub)
                nc.gpsimd.tensor_mul(d2[:, hsl], s_act[:, hsl], d1[:, hsl])

            # devneg (feature-major, Pool)
            devneg_fm = work.tile([D, BC], f32)
            nc.gpsimd.tensor_mul(devneg_fm, devt, _bcast(npt_c, BC))

            # ---------------- E = d1*W2, C = d2*W2 (bf16, Pool) -------------
            e_bf = []
            c_bf = []
            for hc in range(2):
                e_t = work.tile([D, BC * O], bf16, tag=f"e{hc}", name=f"e_t{hc}")
                c_t = work.tile([D, BC * O], bf16, tag=f"c{hc}", name=f"c_t{hc}")
                w2_blk = w2[:, hc * O : (hc + 1) * O]
                w2_view = bass.AP(
                    tensor=w2_blk.tensor, offset=w2_blk.offset,
                    ap=[w2_blk.ap[0], [0, BC], list(w2_blk.ap[1])],
                )
                d1_blk = d1[:, hc * BC : (hc + 1) * BC]
                d1_view = bass.AP(
                    tensor=d1_blk.tensor, offset=d1_blk.offset,
                    ap=[d1_blk.ap[0], list(d1_blk.ap[1]), [0, O]],
                )
                d2_blk = d2[:, hc * BC : (hc + 1) * BC]
                d2_view = bass.AP(
                    tensor=d2_blk.tensor, offset=d2_blk.offset,
                    ap=[d2_blk.ap[0], list(d2_blk.ap[1]), [0, O]],
                )
                nc.gpsimd.tensor_tensor(
                    e_t[:].rearrange("p (b o) -> p b o", b=BC), w2_view, d1_view, mult
                )
                nc.gpsimd.tensor_tensor(
                    c_t[:].rearrange("p (b o) -> p b o", b=BC), w2_view, d2_view, mult
                )
                e_bf.append(e_t)
                c_bf.append(c_t)

            vsq = work.tile([D, BC], f32)
            nc.gpsimd.tensor_mul(vsq, vt, vt)

            # ---------------- F = K@E, KC = K2@C (bf16 matmuls) -------------
            ps_f = [ps_fb[:, 0 : BC * O], ps_fb[:, BC * O : 2 * BC * O]]
            ps_kc = [ps_kb[:, 0 : BC * O], ps_kb[:, BC * O : 2 * BC * O]]
            for mc in range(2):
                for kc in range(2):
                    nc.tensor.matmul(
                        ps_f[mc], k_bf[kc][:, mc * 128 : (mc + 1) * 128], e_bf[kc],
                        start=(kc == 0), stop=(kc == 1), skip_group_check=True,
                    )
            for mc in range(2):
                for kc in range(2):
                    nc.tensor.matmul(
                        ps_kc[mc], k2_bf[kc][:, mc * 128 : (mc + 1) * 128], c_bf[kc],
                        start=(kc == 0), stop=(kc == 1), skip_group_check=True,
                    )
            ps_vnr = ps_tv[0:1, BC : 2 * BC]
            nc.tensor.matmul(ps_vnr, fours_c, vsq, start=True, stop=True, skip_group_check=True)
            vn_sb = work.tile([1, BC], f32)
            nc.vector.tensor_copy(vn_sb, ps_vnr)

            # SBUF bf16 copies of F/KC - single fused copies per bank (two
            # copies of one PSUM tile from different engines would serialize
            # on the tile's accessor chain)
            # F copies split per-mc on DVE so Pool's first mx quarter starts
            # as soon as the mc0 matmuls land; KC (termA only, slack) on ACT
            f_all = work.tile([D, 2 * BC * O], bf16)
            nc.vector.tensor_copy(f_all[:, 0 : BC * O], ps_f[0])
            nc.vector.tensor_copy(f_all[:, BC * O : 2 * BC * O], ps_f[1])
            kc_all = work.tile([D, 2 * BC * O], bf16)
            nc.scalar.copy(kc_all, ps_kb[:, 0 : 2 * BC * O])
            f_bf = [f_all[:, 0 : BC * O], f_all[:, BC * O : 2 * BC * O]]
            kc_bf = [kc_all[:, 0 : BC * O], kc_all[:, BC * O : 2 * BC * O]]

            # ---------------- q path (f32): T = W1 @ g ----------------------
            # dd = [d2p | d1] in one tile so each tq multiply is a single
            # strided-view op against ps_q ([q1 | q2]); all Pool (SBUF only).
            # The d1 half gets a zero term derived from mx00 purely to delay
            # tq's readiness: otherwise the greedy scheduler runs tq (T path,
            # lots of slack) on DVE ahead of the critical mx quarter.
            dd = work.tile([D, 4 * BC], f32)
            d2p = dd[:, 0 : 2 * BC]
            nc.gpsimd.tensor_mul(d2p, d2, p_t)
            # qrhs gains a zero term derived from f_all purely to delay the
            # downstream tq op's readiness: otherwise the greedy scheduler
            # runs tq (T path, big slack) on DVE ahead of the critical mx
            # quarter and delays Y by ~200ns.
            qt0 = work.tile([D, 2 * BC], f32)
            nc.gpsimd.tensor_mul(qt0, d1, p_t)
            qt1 = work.tile([D, 2 * BC], f32)
            nc.gpsimd.tensor_mul(qt1, d2p, p_t)
            zf = work.tile([D, 2 * BC], f32)
            nc.gpsimd.tensor_sub(zf, f_all[:, 0 : 2 * BC], f_all[:, 0 : 2 * BC])
            qrhs = work.tile([D, 4 * BC], bf16)  # cols [d1p(8) | d2pp(8)]
            nc.gpsimd.tensor_add(qrhs[:, 0 : 2 * BC], qt0, zf)
            nc.gpsimd.tensor_add(qrhs[:, 2 * BC : 4 * BC], qt1, zf)

            qv = qrhs[:].rearrange("p (g c b) -> p c g b", g=2, c=2, b=BC)
            # q = S @ [d1p | d2pp] directly (S = W2 W2^T from host, bf16)
            ps_q = [ps_zq[:, 16 : 16 + 2 * BC], ps_zq[:, 16 + 2 * BC : 16 + 4 * BC]]
            for mc in range(2):
                for kc in range(2):
                    nc.tensor.matmul(
                        ps_q[mc], s_bf[kc][:, mc * 128 : (mc + 1) * 128], qv[:, kc],
                        start=(kc == 0), stop=(kc == 1), skip_group_check=True,
                    )
            # ---------------- mx[hc][:, (b,o,o')] = F[:,b,o]*C[:,b,o'] ------
            # 4 separate tiles (kills write-after-read coupling vs Y reads);
            # Pool reads F straight from PSUM (no access penalty), the DVE
            # quarter reads the f_bf SBUF copy instead.
            half = BC // 2 * OO  # 200
            mxq = {}
            for hf in range(2):
                for hc in range(2):
                    mxq[(hc, hf)] = work.tile(
                        [D, half], bf16, tag=f"mx{hc}{hf}", name=f"mx{hc}{hf}"
                    )
            for hf in range(2):
                for hc in range(2):
                    on_dve = hc == 1 and hf == 0
                    f_src = f_bf[hc]
                    f_half = f_src[:, hf * 2 * O : (hf * 2 + 2) * O]
                    c_half = c_bf[hc][:, hf * 2 * O : (hf * 2 + 2) * O]
                    f3 = f_half.rearrange("p (b a) -> p b a", b=2)
                    c3 = c_half.rearrange("p (b a) -> p b a", b=2)
                    f_view = bass.AP(
                        tensor=f3.tensor, offset=f3.offset,
                        ap=[f3.ap[0], list(f3.ap[1]), list(f3.ap[2]), [0, O]],
                    )
                    c_view = bass.AP(
                        tensor=c3.tensor, offset=c3.offset,
                        ap=[c3.ap[0], list(c3.ap[1]), [0, O], list(c3.ap[2])],
                    )
                    out_v = mxq[(hc, hf)][:].rearrange("p (b a c) -> p b a c", b=2, a=O)
                    eng = nc.vector if on_dve else nc.gpsimd
                    eng.tensor_tensor(out_v, f_view, c_view, mult)

            nc.gpsimd.tensor_copy(dd[:, 2 * BC : 4 * BC], d1)

            # tq cols [t1(mc,b) 0:8 | t2(mc,b) 8:16]: one DVE op reading all
            # of ps_q via 4D views (gpsimd may not read PSUM)
            tq = work.tile([D, 4 * BC], bf16)
            dd_v = dd[:].rearrange("p (g c b) -> p c g b", g=2, c=2, b=BC)
            tq_v = tq[:].rearrange("p (g c b) -> p c g b", g=2, c=2, b=BC)
            q_all = ps_zq[:, 16:32].rearrange("p (c g b) -> p c g b", c=2, g=2, b=BC)
            nc.vector.tensor_tensor(tq_v, dd_v, q_all, mult)
            for i, (hc, csl) in enumerate(
                [(0, slice(0, BC)), (1, slice(BC, 2 * BC)),
                 (0, slice(2 * BC, 3 * BC)), (1, slice(3 * BC, 4 * BC))]
            ):
                nc.tensor.matmul(ps_Tm, w1t_bf[hc], tq[:, csl], start=(i == 0), stop=(i == 3))
            T_sb = work.tile([D, BC], f32)
            nc.vector.tensor_copy(T_sb, ps_Tm)

            # ---------------- Y = W1 @ mx (bf16), termB ---------------------
            # Y must leave PSUM before the swap-product (an instruction may
            # read at most one PSUM operand): per half, copy the two sample
            # blocks out on DVE and ACT in parallel, then Pool does the
            # product+reduce from SBUF.
            junk_b = work.tile([D, BC * OO], f32)
            tb = work.tile([D, 2], f32)
            tb_row = work.tile([1, 2], f32)
            y_sb = work.tile([D, BC * OO], f32)
            for hf in range(2):
                ps_y = ps.tile([D, half], f32, tag=f"Y{hf}", bufs=1, name=f"ps_y{hf}")
                for hc in range(2):
                    nc.tensor.matmul(
                        ps_y, w1t_bf[hc], mxq[(hc, hf)],
                        start=(hc == 0), stop=(hc == 1),
                    )
                if hf == 1:
                    nc.vector.tensor_copy(y_sb[:, half : 2 * half], ps_y)
                else:
                    nc.scalar.copy(y_sb[:, 0:half], ps_y)
                for bi in range(2):
                    b = hf * 2 + bi
                    blk = y_sb[:, b * OO : (b + 1) * OO]
                    v1 = blk.rearrange("p (a c) -> p a c", a=O)
                    v2 = blk.rearrange("p (a c) -> p c a", a=O, c=O)
                    jv = junk_b[:, b * OO : (b + 1) * OO].rearrange("p (a c) -> p a c", a=O)
                    if hf == 1:
                        # DVE: product+reduce in one STT (SBUF operands)
                        nc.vector.scalar_tensor_tensor(
                            jv, v1, 1.0, v2, mult, mult, accum_out=tb[:, b - 2 : b - 1]
                        )
                    else:
                        # Pool: TT product + full (XYZWC) reduce to [1,1]
                        nc.gpsimd.tensor_tensor(jv, v1, v2, mult)
                        nc.gpsimd.tensor_reduce(
                            tb_row[:, b : b + 1],
                            junk_b[:, b * OO : (b + 1) * OO],
                            mybir.AxisListType.XYZWC, mybir.AluOpType.add,
                        )

            # ---------------- termA: G1 = E^T F, G2 = C^T KC ----------------
            # all 16 matmuls first, then the 4 reductions (no PE<->Pool ping-pong)
            junk_a = work.tile([O, BC * O], f32)
            for b in range(BC):
                ps_g = ps_gt[:, b * 2 * O : (b + 1) * 2 * O]
                bsl = slice(b * O, (b + 1) * O)
                for hc in range(2):
                    nc.tensor.matmul(
                        ps_g[:, 0:O], e_bf[hc][:, bsl], f_bf[hc][:, bsl],
                        start=(hc == 0), stop=(hc == 1), skip_group_check=True,
                    )
                for hc in range(2):
                    nc.tensor.matmul(
                        ps_g[:, O : 2 * O], c_bf[hc][:, bsl], kc_bf[hc][:, bsl],
                        start=(hc == 0), stop=(hc == 1), skip_group_check=True,
                    )
            g_sb = work.tile([O, BC * 2 * O], f32)
            nc.scalar.copy(g_sb, ps_gt[:, 0 : BC * 2 * O])
            ta_row = work.tile([1, BC], f32)
            for b in range(BC):
                g_blk = g_sb[:, b * 2 * O : (b + 1) * 2 * O]
                nc.gpsimd.tensor_mul(
                    junk_a[:, b * O : (b + 1) * O], g_blk[:, 0:O], g_blk[:, O : 2 * O]
                )
                nc.gpsimd.tensor_reduce(
                    ta_row[:, b : b + 1], junk_a[:, b * O : (b + 1) * O],
                    mybir.AxisListType.XYZWC, mybir.AluOpType.add,
                )

            # ---------------- r4 = 2*(termA+termB); rsc = -0.5*(r4*vsq)^-.5 -
            ps_r4r = ps_tv[0:1, 0:BC]
            nc.tensor.matmul(ps_r4r[:, 0:2], twos_c, tb, start=True, stop=False, skip_group_check=True)
            nc.tensor.matmul(ps_r4r[:, 2:4], twos_c[0:1], tb_row, start=True, stop=False, skip_group_check=True)
            nc.tensor.matmul(ps_r4r[:, 0:2], twos_c[0:1], ta_row[:, 0:2], start=False, stop=True, skip_group_check=True)
            nc.tensor.matmul(ps_r4r[:, 2:4], twos_c[0:1], ta_row[:, 2:4], start=False, stop=True, skip_group_check=True)
            # rsc = (2(ta+tb) * 4|v|^2)^-0.5 = +0.5/(nf*|v|); the sign of a
            # is carried by T (d2 = s*d1 flips/rescales T, norm path absorbs
            # its square). No sqrt/pow in the ISA: quake rsqrt (bit-hack seed
            # + one Newton step) in cheap Pool TT ops, rel err <2e-3.
            u32 = mybir.dt.uint32
            m_sb = work.tile([1, BC], f32)
            nc.vector.tensor_mul(m_sb, ps_r4r, vn_sb)
            sh = work.tile([1, BC], f32)
            nc.vector.tensor_tensor(
                sh[:].bitcast(u32), m_sb[:].bitcast(u32), _bcast(oneb_c[0:1, :], BC).bitcast(u32),
                mybir.AluOpType.logical_shift_right,
            )
            mh = work.tile([1, BC], f32)
            nc.gpsimd.tensor_mul(mh, m_sb, _bcast(cnh_c[0:1, :], BC))  # -0.5*m
            y0 = work.tile([1, BC], f32)
            nc.gpsimd.tensor_tensor(
                y0[:].bitcast(u32), _bcast(magic_c[0:1, :], BC).bitcast(u32), sh[:].bitcast(u32),
                mybir.AluOpType.subtract,
            )
            yy = work.tile([1, BC], f32)
            nc.gpsimd.tensor_mul(yy, y0, y0)
            hyy = work.tile([1, BC], f32)
            nc.gpsimd.tensor_mul(hyy, yy, mh)
            un = work.tile([1, BC], f32)
            nc.gpsimd.tensor_add(un, hyy, _bcast(c15_c[0:1, :], BC))
            rsc = work.tile([1, BC], f32)
            nc.gpsimd.tensor_mul(rsc, y0, un)
            rsc_bc = work.tile([D, BC], f32)
            nc.gpsimd.partition_broadcast(rsc_bc, rsc)

            # ------- out (feature-major) = T*rsc + devneg; transposed DMA ---
            tr = work.tile([D, BC], f32)
            nc.gpsimd.tensor_mul(tr, T_sb, rsc_bc)
            out_fm = work.tile([D, BC], f32)
            nc.gpsimd.tensor_add(out_fm, tr, devneg_fm)
            nc.sync.dma_start(out=acc_d[:].rearrange("b d -> d b"), in_=out_fm)

    nc.finalize()
    return nc


def _get_program():
    global _PROGRAM
    if _PROGRAM is None:
        _PROGRAM = _build_program()
    return _PROGRAM


def make_in_maps(t, state_batch, x0, x1, W1, b1, W2):
    import ml_dtypes

    bf = ml_dtypes.bfloat16
    W1 = np.ascontiguousarray(np.asarray(W1, np.float32))
    b1 = np.asarray(b1, np.float32)
    W2 = np.asarray(W2, np.float32)
    tt = float(np.asarray(t).ravel()[0])
    win = 4.0 * tt * (1.0 - tt)

    K64 = np.asarray(W1, np.float64).T @ np.asarray(W1, np.float64)
    K = K64.astype(np.float32)
    K2 = (K64 * K64).astype(np.float32)
    kb = np.ascontiguousarray(
        np.concatenate([K[0:128, :], K[128:256, :]], axis=1).astype(bf)
    )
    S64 = np.asarray(W2, np.float64) @ np.asarray(W2, np.float64).T
    S = S64.astype(np.float32)
    sbm = np.zeros((128, 512), np.float32)
    sbm[:, 0:256] = S[0:128, :]
    sbm[:, 256:512] = S[128:256, :]
    kw = np.zeros((128, 768), np.float32)
    kw[:, 0:256] = K2[0:128]
    kw[:, 256:512] = K2[128:256]
    kw[:, 512:640] = W1[:, 0:128].T
    kw[:, 640:768] = W1[:, 128:256].T
    kw = np.ascontiguousarray(kw.astype(bf))

    xw = np.empty((128, 304), np.float32)
    xw[:, 300] = -0.1
    xw[:, 301:304] = 0.0
    xw[:, 296] = np.uint32(0x5F3759DF).view(np.float32) if False else np.frombuffer(
        np.uint32(0x5F3759DF).tobytes(), dtype=np.float32
    )[0]
    xw[:, 297] = np.frombuffer(np.uint32(1).tobytes(), dtype=np.float32)[0]
    xw[:, 298] = 1.5
    xw[:, 299] = -0.5
    xw[:, 18:274] = W1
    xw[:, 274:276] = b1.reshape(2, 128).T
    xw[:, 276:296] = W2.reshape(2, 128, O).transpose(1, 0, 2).reshape(128, 2 * O)

    dev = np.asarray(state_batch[:B], np.float32)
    v = np.asarray(state_batch[B:], np.float32)
    x0 = np.asarray(x0, np.float32)
    x1 = np.asarray(x1, np.float32)
    in_maps = []
    for c in range(NCORES):
        sl = slice(c * BC, (c + 1) * BC)
        xwc = xw.copy()
        xwc[:, 0:BC] = dev[sl].T
        xwc[:, BC : 2 * BC] = x0[sl].T
        xwc[:, 2 * BC : 3 * BC] = x1[sl].T
        xwc[:, 3 * BC : 4 * BC] = v[sl].T
        xwc[:, 16] = tt
        xwc[:, 17] = win
        in_maps.append(
            {"xw": np.ascontiguousarray(xwc), "kb": kb,
             "sb": np.ascontiguousarray(sbm.astype(bf)), "kw": kw}
        )
    return in_maps


def kernel(t, state_batch, x0, x1, W1, b1, W2, b2):
    from concourse import bass_utils

    t = np.asarray(t)
    state_batch = np.asarray(state_batch)
    x0 = np.asarray(x0)
    x1 = np.asarray(x1)
    W1 = np.asarray(W1)
    b1 = np.asarray(b1)
    W2 = np.asarray(W2)

    nc = _get_program()
    in_maps = make_in_maps(t, state_batch, x0, x1, W1, b1, W2)
    res = bass_utils.run_bass_kernel_spmd(nc, in_maps, core_ids=list(range(NCORES)))
    acc = np.concatenate([res.results[c]["acc"] for c in range(NCORES)], axis=0)
    v = state_batch[B:].astype(np.float32)
    return np.concatenate([v, acc.astype(np.float32)], axis=0)
